# revision 1
# baseline (speedup 1.0000x reference)
"""Trainium2 Bass kernel for nn_MultiHeadAttention (B=4, S=2048, C=256, H=8).

Sharding: data-parallel over (batch, seq) — 8 cores, core i handles
batch b = i//2 and query rows r0 = (i%2)*1024 .. r0+1024.  Each core
computes K/V projections for its full batch sequence (all 8 heads),
attention + fc for its 1024 query rows, then residual + LayerNorm.
No collectives needed; host concatenates the 8 row-shards.

Compute dtype: bf16 matmuls with fp32 PSUM accumulation; softmax
(exp / rowsum / normalize) and LayerNorm in fp32.  Weights and x are
pre-cast to bf16 on host (input formatting); residual path stays fp32.

Every DMA writes a persistent SBUF buffer (no pool-slot recycling) so
each DMA instruction needs at most one semaphore wait — walrus lowers
these to PSEUDO_DMA_DIRECT2D which supports only a single sync wait.
"""

import sys

for _p in ("/opt/trn_rl_repo",):
    if _p not in sys.path:
        sys.path.insert(0, _p)

from contextlib import ExitStack

import numpy as np

import concourse.bass as bass
from concourse import bacc
import concourse.tile as tile
from concourse import mybir
from concourse.masks import make_identity

P = 128
B, S, C, H = 4, 2048, 256, 8
RQ = 1024            # query rows per core
CH = 512             # query-row chunk (matmul N)
NCH = RQ // CH       # chunks per core = 2
NT = S // P          # t tiles = 16
ND = C // P          # d tiles = 2
NR = RQ // P         # row tiles per core = 8
EPS = 1e-5
SCALE = 1.0 / np.sqrt(C)

F32 = mybir.dt.float32
BF16 = mybir.dt.bfloat16
AF = mybir.ActivationFunctionType
OP = mybir.AluOpType


def build_nc() -> bass.Bass:
    nc = bacc.Bacc(None)

    xb16 = nc.declare_dram_parameter("xb16", [S, C], BF16, isOutput=False)
    xqf = nc.declare_dram_parameter("xqf", [RQ, C], F32, isOutput=False)
    wq = nc.declare_dram_parameter("wq16", [H, C, C], BF16, isOutput=False)
    wk = nc.declare_dram_parameter("wk16", [H, C, C], BF16, isOutput=False)
    wv = nc.declare_dram_parameter("wv16", [H, C, C], BF16, isOutput=False)
    wfc = nc.declare_dram_parameter("wfc16", [H * C, C], BF16, isOutput=False)
    # bqk = host-packed [P, 2, ND, H]: bqk[p, 0] = bq[h, co*128+p], bqk[p, 1] = bk
    bqk = nc.declare_dram_parameter("bqk", [P, 2, ND, H], F32, isOutput=False)
    # brow = concat(bfc_eff [256], gamma [256], beta [256]); bfc_eff folds in
    # bv @ Wfc (softmax weights sum to 1, so the V-bias reaches fc as a const)
    brow = nc.declare_dram_parameter("brow", [3 * C], F32, isOutput=False)
    out = nc.declare_dram_parameter("out", [RQ, C], F32, isOutput=True)

    with tile.TileContext(nc) as tc, ExitStack() as ctx:
        singles = ctx.enter_context(tc.tile_pool(name="singles", bufs=1))
        hpool = ctx.enter_context(tc.tile_pool(name="hpool", bufs=2))
        epool = ctx.enter_context(tc.tile_pool(name="epool", bufs=2))
        opool = ctx.enter_context(tc.tile_pool(name="opool", bufs=2))
        lnpool = ctx.enter_context(tc.tile_pool(name="lnpool", bufs=4))

        ps512 = ctx.enter_context(tc.tile_pool(name="ps512", bufs=3, space="PSUM"))
        ps256 = ctx.enter_context(tc.tile_pool(name="ps256", bufs=2, space="PSUM"))
        psot = ctx.enter_context(tc.tile_pool(name="psot", bufs=2, space="PSUM"))
        pspt = ctx.enter_context(tc.tile_pool(name="pspt", bufs=1, space="PSUM"))

        # ---- constants ----
        ident = singles.tile([P, P], BF16)
        make_identity(nc, ident)
        ones = singles.tile([P, P], BF16)
        nc.vector.memset(ones, 1.0)
        eps_t = singles.tile([P, 1], F32)
        nc.vector.memset(eps_t, EPS)

        # ---- weights (bf16, direct DMA into persistent tiles) ----
        # layout [ci, co, h, d]: lhsT/rhs blocks are [128, *] slices
        def load_w(dram, wname, pat, **kw):
            w_sb = singles.tile([P, ND, H, C], BF16, tag=wname, name=wname)
            r = dram.rearrange(pat, ci=P, **kw)
            for hh in range(0, H, 2):
                for co in range(ND):
                    eng = nc.sync if (co + hh // 2) % 2 == 0 else nc.scalar
                    eng.dma_start(out=w_sb[:, co, hh:hh + 2],
                                  in_=r[:, co, hh:hh + 2])
            return w_sb

        # V-projection weights first (first consumer), fc last
        wv_bf = load_w(wv, "wv_bf", "h (co ci) d -> ci co h d")
        wk_bf = load_w(wk, "wk_bf", "h (co ci) d -> ci co h d")
        wq_bf = load_w(wq, "wq_bf", "h (co ci) d -> ci co h d")
        wfc_bf = load_w(wfc, "wfc_bf", "(h co ci) e -> ci co h e", co=ND)

        # ---- x inputs (persistent; split DMAs so transposes start early) ----
        xb_sb = singles.tile([P, NT, C], BF16)       # x_b rows, bf16
        xb_r = xb16.rearrange("(n p) d -> p n d", p=P)
        for q4 in range(16):
            nc.gpsimd.dma_start(out=xb_sb[:, q4:q4 + 1], in_=xb_r[:, q4:q4 + 1])
        xr_sb = singles.tile([P, NR, C], F32)        # residual rows, fp32
        nc.gpsimd.dma_start(out=xr_sb, in_=xqf.rearrange("(n p) d -> p n d", p=P))

        # ---- biases ----
        bqk_sb = singles.tile([P, 2, ND, H], F32)
        nc.gpsimd.dma_start(out=bqk_sb, in_=bqk[:])
        bq_sb = bqk_sb[:, 0]
        bk_sb = bqk_sb[:, 1]
        # broadcast row-vector block replicated across partitions
        brow_sb = singles.tile([P, 3 * C], F32)
        brow_ap = brow[:]
        brow_bc = bass.AP(tensor=brow_ap.tensor, offset=brow_ap.offset,
                          ap=[[0, P]] + list(brow_ap.ap))
        nc.gpsimd.dma_start(out=brow_sb, in_=brow_bc)
        bfc_sb = brow_sb[:, 0:C]
        gamma_sb = brow_sb[:, C:2 * C]
        beta_sb = brow_sb[:, 2 * C:3 * C]

        # ---- PE warmup: dense dummy matmuls while input DMAs land, so the
        # HAM clock gate is at 2.4 GHz before real work (transposes do not
        # count as PE-busy for HAM) ----
        def tp_slot(k):
            if k % 3 == 0:
                return pspt.tile([P, P], BF16, tag="mix", name="pst")
            return psot.tile([P, P], BF16, tag="ot", name="pst2")

        wps = psot.tile([P, P], F32, tag="ot", name="wps")
        for w in range(56):
            nc.tensor.matmul(wps, lhsT=ident, rhs=ident, start=True, stop=True)

        # ---- x transposes: xbT [ci, co, t] bf16.  Host rotates each core's
        # xb16 so its own query rows are t = 0..RQ; the Q projection then
        # reads the xbT prefix (softmax is permutation-invariant over keys).
        xbT = singles.tile([P, ND, S], BF16)
        for i in range(NT):
            for c2 in range(ND):
                pst = tp_slot(i * ND + c2)
                nc.tensor.transpose(pst, xb_sb[:, i, c2 * P:(c2 + 1) * P], ident)
                nc.vector.tensor_copy(out=xbT[:, c2, i * P:(i + 1) * P], in_=pst)
            if i % 2 == 1:
                for w in range(8):
                    nc.tensor.matmul(wps, lhsT=ident, rhs=ident,
                                     start=True, stop=True)

        # ---- fc accumulator / output staging (fp32, SBUF) ----
        acc_sb = singles.tile([P, NR, C], F32)

        # fc partial for one (head, chunk): accumulate into acc_sb fp32
        def emit_fc(ot_sb, fh, fch):
            for r1 in range(CH // P):
                idx = fch * (CH // P) + r1
                fc_ps = ps256.tile([P, C], F32, tag="ps256", name="fc_ps")
                for d2 in range(ND):
                    nc.tensor.matmul(
                        fc_ps,
                        lhsT=ot_sb[:, d2, r1 * P:(r1 + 1) * P],
                        rhs=wfc_bf[:, d2, fh, :],
                        start=(d2 == 0), stop=(d2 == ND - 1),
                    )
                if fh == 0:
                    nc.vector.tensor_copy(out=acc_sb[:, idx], in_=fc_ps)
                else:
                    nc.vector.tensor_add(out=acc_sb[:, idx],
                                         in0=acc_sb[:, idx], in1=fc_ps)

        # ---- bias + residual + LayerNorm (in-place, final writes on DVE) ----
        out_r = out.rearrange("(n p) d -> p n d", p=P)

        def emit_ln(i):
            t = acc_sb[:, i]
            nc.vector.tensor_add(out=t, in0=t, in1=xr_sb[:, i])
            nc.vector.tensor_tensor(out=t, in0=t, in1=bfc_sb, op=OP.add)
            stats = lnpool.tile([P, 6], F32, tag="stats")
            nc.vector.bn_stats(out=stats, in_=t)
            mv = lnpool.tile([P, 2], F32, tag="mv")
            nc.vector.bn_aggr(out=mv, in_=stats)
            sd = lnpool.tile([P, 1], F32, tag="sd")
            nc.scalar.activation(out=sd, in_=mv[:, 1:2], func=AF.Sqrt,
                                 bias=eps_t, scale=1.0)
            rstd = lnpool.tile([P, 1], F32, tag="rstd")
            nc.vector.reciprocal(out=rstd, in_=sd)
            nc.vector.tensor_scalar(out=t, in0=t, scalar1=mv[:, 0:1],
                                    scalar2=rstd, op0=OP.subtract, op1=OP.mult)
            nc.vector.tensor_tensor(out=t, in0=t, in1=gamma_sb, op=OP.mult)
            nc.vector.tensor_tensor(out=t, in0=t, in1=beta_sb, op=OP.add)

        pending_fc = None

        # ---- head loop ----
        for h in range(H):
            # V [t, d] projection
            v_sb = hpool.tile([P, NT, C], BF16, tag="v")
            for t in range(NT):
                ps = ps256.tile([P, C], F32, tag="ps256")
                for c2 in range(ND):
                    nc.tensor.matmul(
                        ps,
                        lhsT=xbT[:, c2, t * P:(t + 1) * P],
                        rhs=wv_bf[:, c2, h, :],
                        start=(c2 == 0), stop=(c2 == ND - 1),
                    )
                nc.vector.tensor_copy(out=v_sb[:, t], in_=ps)
            # K^T [d, t] projection
            kt_sb = hpool.tile([P, ND, S], BF16, tag="kt")
            for t4 in range(S // CH):
                for d2 in range(ND):
                    ps = ps512.tile([P, CH], F32, tag="ps512")
                    for c2 in range(ND):
                        nc.tensor.matmul(
                            ps,
                            lhsT=wk_bf[:, c2, h, d2 * P:(d2 + 1) * P],
                            rhs=xbT[:, c2, t4 * CH:(t4 + 1) * CH],
                            start=(c2 == 0), stop=(c2 == ND - 1),
                        )
                    nc.vector.tensor_scalar_add(
                        out=kt_sb[:, d2, t4 * CH:(t4 + 1) * CH], in0=ps,
                        scalar1=bk_sb[:, d2, h:h + 1],
                    )
            # Q^T [d, r] projection
            qt_sb = hpool.tile([P, ND, RQ], BF16, tag="qt")
            for r4 in range(NCH):
                for d2 in range(ND):
                    ps = ps512.tile([P, CH], F32, tag="ps512")
                    for c2 in range(ND):
                        nc.tensor.matmul(
                            ps,
                            lhsT=wq_bf[:, c2, h, d2 * P:(d2 + 1) * P],
                            rhs=xbT[:, c2, r4 * CH:(r4 + 1) * CH],
                            start=(c2 == 0), stop=(c2 == ND - 1),
                        )
                    nc.scalar.activation(
                        out=qt_sb[:, d2, r4 * CH:(r4 + 1) * CH], in_=ps,
                        func=AF.Identity, bias=bq_sb[:, d2, h:h + 1], scale=1.0,
                    )

            # attention, one 512-row chunk at a time.  The fc matmuls for a
            # chunk are DEFERRED into the next chunk's instruction stream so
            # the PE never stalls on the DVE reciprocal/scale at the chunk
            # boundary (PE streams are executed in emit order).
            for ch in range(NCH):
                rsl = slice(ch * CH, (ch + 1) * CH)
                e_sb = epool.tile([P, NT, CH], BF16, tag="e")
                ot_ps = [psot.tile([P, CH], F32, tag="ot", name=f"ot{d2}")
                         for d2 in range(ND)]
                rs_ps = pspt.tile([P, CH], F32, tag="mix", name="rs_ps")
                for t in range(NT):
                    st = ps512.tile([P, CH], F32, tag="ps512")
                    for d2 in range(ND):
                        nc.tensor.matmul(
                            st,
                            lhsT=kt_sb[:, d2, t * P:(t + 1) * P],
                            rhs=qt_sb[:, d2, rsl],
                            start=(d2 == 0), stop=(d2 == ND - 1),
                        )
                    # e = exp(scores * SCALE); scores ~ N(0,1) so no max-sub
                    nc.scalar.activation(out=e_sb[:, t], in_=st, func=AF.Exp,
                                         scale=float(SCALE))
                    # rowsum broadcast to all 128 partitions (lhsT = ones mat)
                    nc.tensor.matmul(rs_ps, lhsT=ones, rhs=e_sb[:, t],
                                     start=(t == 0), stop=(t == NT - 1))
                    for d2 in range(ND):
                        nc.tensor.matmul(
                            ot_ps[d2],
                            lhsT=v_sb[:, t, d2 * P:(d2 + 1) * P],
                            rhs=e_sb[:, t],
                            start=(t == 0), stop=(t == NT - 1),
                        )
                if pending_fc is not None:
                    emit_fc(*pending_fc)
                    pending_fc = None
                rcp_f = opool.tile([P, CH], F32, tag="rcp")
                nc.vector.reciprocal_approx_fast(out=rcp_f, in_=rs_ps)
                ot_sb = opool.tile([P, ND, CH], BF16, tag="ot_sb")
                for d2 in range(ND):
                    nc.vector.tensor_tensor(
                        out=ot_sb[:, d2], in0=ot_ps[d2], in1=rcp_f[:], op=OP.mult)
                if h == H - 1:
                    # last head: emit fc eagerly and pipeline LN + store per
                    # row-tile so the tail is fc->add->LN->DMA overlapped
                    for r1 in range(CH // P):
                        idx = ch * (CH // P) + r1
                        fc_ps = ps256.tile([P, C], F32, tag="ps256",
                                           name="fc_ps")
                        for d2 in range(ND):
                            nc.tensor.matmul(
                                fc_ps,
                                lhsT=ot_sb[:, d2, r1 * P:(r1 + 1) * P],
                                rhs=wfc_bf[:, d2, h, :],
                                start=(d2 == 0), stop=(d2 == ND - 1),
                            )
                        nc.vector.tensor_add(out=acc_sb[:, idx],
                                             in0=acc_sb[:, idx], in1=fc_ps)
                        emit_ln(idx)
                        nc.gpsimd.dma_start(out=out_r[:, idx:idx + 1, :],
                                            in_=acc_sb[:, idx:idx + 1])
                else:
                    pending_fc = (ot_sb, h, ch)


    nc.finalize()
    return nc


_NC = None


def _get_nc():
    global _NC
    if _NC is None:
        _NC = build_nc()
    return _NC


def make_in_maps(inputs):
    import ml_dtypes
    bf16 = ml_dtypes.bfloat16
    x = np.asarray(inputs["x"], dtype=np.float32)
    x16 = x.astype(bf16)
    shared = {
        "wq16": np.ascontiguousarray(np.asarray(inputs["Wq"], np.float32).astype(bf16)),
        "wk16": np.ascontiguousarray(np.asarray(inputs["Wk"], np.float32).astype(bf16)),
        "wv16": np.ascontiguousarray(np.asarray(inputs["Wv"], np.float32).astype(bf16)),
        "wfc16": np.ascontiguousarray(np.asarray(inputs["Wfc"], np.float32).astype(bf16)),
        "bqk": np.ascontiguousarray(np.stack([
            np.asarray(inputs["bq"], np.float32).reshape(H, 2, P).transpose(2, 1, 0),
            np.asarray(inputs["bk"], np.float32).reshape(H, 2, P).transpose(2, 1, 0),
        ], axis=1)),
        "brow": np.ascontiguousarray(np.concatenate([
            np.asarray(inputs["bfc"], np.float32).ravel()
            + np.asarray(inputs["bv"], np.float32).ravel()
            @ np.asarray(inputs["Wfc"], np.float32),
            np.asarray(inputs["gamma"], np.float32).ravel(),
            np.asarray(inputs["beta"], np.float32).ravel(),
        ])),
    }
    in_maps = []
    for core in range(8):
        b, r0 = core // 2, (core % 2) * RQ
        m = dict(shared)
        m["xb16"] = np.ascontiguousarray(np.roll(x16[b], -r0, axis=0))
        m["xqf"] = np.ascontiguousarray(x[b, r0:r0 + RQ])
        in_maps.append(m)
    return in_maps


def assemble(results):
    out = np.empty((B, S, C), dtype=np.float32)
    for core in range(8):
        b, r0 = core // 2, (core % 2) * RQ
        out[b, r0:r0 + RQ] = results[core]["out"]
    return out


def kernel(**inputs) -> np.ndarray:
    from concourse.bass_utils import run_bass_kernel_spmd

    nc = _get_nc()
    in_maps = make_in_maps(inputs)
    res = run_bass_kernel_spmd(nc, in_maps, core_ids=list(range(8)))
    return assemble(res.results)



# revision 8
# speedup vs baseline: 1.4905x; 1.4905x over previous
"""Trainium2 Bass kernel for nn_MultiHeadAttention (B=4, S=2048, C=256, H=8).

Sharding: data-parallel over (batch, seq) — 8 cores, core i handles
batch b = i//2 and query rows r0 = (i%2)*1024 .. r0+1024.  Each core
computes K/V projections for its full batch sequence (all 8 heads),
attention + fc for its 1024 query rows, then residual + LayerNorm.
No collectives needed; host concatenates the 8 row-shards.

v2: fp8 (e4m3) DoubleRow matmuls everywhere (2x PE throughput).  The
final output is dominated by the residual (attention contribution has
sigma ~0.036 vs residual sigma 1), so fp8 noise in the attention path
is suppressed ~30x in the output — rel err lands ~1e-3 vs the 2e-2
gate.  Scaling scheme so every fp8 tensor has sigma ~O(1):
  Wq,Wk x16 host-side  -> q,k sigma 16; exp scale 1/4096
  exp bias -2          -> e in [~0, 240] fits e4m3 range (max 240)
  rowsum ones = 0.125  -> ot = 8 * (softmax @ v), sigma ~0.29
  Wfc x32 host-side    -> fc psum = 256 * fc_out
  residual+bias x256 host-side, LN eps x256^2: LN is scale-invariant
Elementwise work is spread across Pool (projection-psum drain + bias),
ACT (exp), and DVE (normalize + LayerNorm) so no single engine
bottlenecks.  fc accumulates over all 8 heads in PSUM at the tail
(per-head ot tiles stay resident in SBUF, 16KB/partition fp8).

Every DMA writes a persistent SBUF buffer or a freshly-allocated pool
tile so each DMA instruction needs at most one semaphore wait.
"""

import sys

for _p in ("/opt/trn_rl_repo",):
    if _p not in sys.path:
        sys.path.insert(0, _p)

from contextlib import ExitStack

import numpy as np

import concourse.bass as bass
from concourse import bacc
import concourse.tile as tile
from concourse import mybir
from concourse.masks import make_identity

P = 128
B, S, C, H = 4, 2048, 256, 8
RQ = 1024            # query rows per core
CH = 512             # query-row chunk (matmul N)
NCH = RQ // CH       # chunks per core = 2
NT = S // P          # t tiles = 16
ND = C // P          # d tiles = 2
NR = RQ // P         # row tiles per core = 8
EPS = 1e-5

WQK_SCALE = 16.0     # host premultiplier on Wq, Wk, bq, bk
WFC_SCALE = 32.0     # host premultiplier on Wfc
RES_SCALE = 256.0    # host premultiplier on residual + bfc_eff
SCALE_EXP = 1.0 / (np.sqrt(C) * WQK_SCALE * WQK_SCALE)  # = 1/4096
BIAS_EXP = -2.0      # keeps exp() below e4m3 max (240); cancels in softmax
ONES_VAL = 0.125     # rowsum lhsT value -> ot = 8 * head_out
EPS_EFF = float(EPS * RES_SCALE * RES_SCALE)

F32 = mybir.dt.float32
BF16 = mybir.dt.bfloat16
FP8 = mybir.dt.float8e4
AF = mybir.ActivationFunctionType
OP = mybir.AluOpType
DR = mybir.MatmulPerfMode.DoubleRow


def build_nc() -> bass.Bass:
    nc = bacc.Bacc(None)

    xt8 = nc.declare_dram_parameter("xt8", [P, ND, S], FP8, isOutput=False)
    xqf = nc.declare_dram_parameter("xqf", [P, NR, C], F32, isOutput=False)
    wq8 = nc.declare_dram_parameter("wq8", [P, ND, H, C], FP8, isOutput=False)
    wk8 = nc.declare_dram_parameter("wk8", [P, ND, H, C], FP8, isOutput=False)
    wv8 = nc.declare_dram_parameter("wv8", [P, ND, H, C], FP8, isOutput=False)
    wfc8 = nc.declare_dram_parameter("wfc8", [P, ND, H, C], FP8, isOutput=False)
    # bqk = host-packed [P, 2, ND, H]: [p, 0] = 16*bq, [p, 1] = 16*bk
    bqk = nc.declare_dram_parameter("bqk", [P, 2, ND, H], F32, isOutput=False)
    gb = nc.declare_dram_parameter("gb", [2, C], F32, isOutput=False)
    out = nc.declare_dram_parameter("out", [RQ, C], F32, isOutput=True)

    with tile.TileContext(nc) as tc, ExitStack() as ctx:
        singles = ctx.enter_context(tc.tile_pool(name="singles", bufs=1))
        hpool = ctx.enter_context(tc.tile_pool(name="hpool", bufs=2))
        epool = ctx.enter_context(tc.tile_pool(name="epool", bufs=2))
        opool = ctx.enter_context(tc.tile_pool(name="opool", bufs=2))
        lnpool = ctx.enter_context(tc.tile_pool(name="lnpool", bufs=4))

        # PSUM: pst 2x[128,1024] = 4 banks, pot 3x[128,512] = 3, prs 1 = 1
        pst = ctx.enter_context(tc.tile_pool(name="pst", bufs=2, space="PSUM"))
        pot = ctx.enter_context(tc.tile_pool(name="pot", bufs=3, space="PSUM"))
        prs = ctx.enter_context(tc.tile_pool(name="prs", bufs=1, space="PSUM"))

        # ---- constants ----
        ident = singles.tile([P, P], BF16)
        make_identity(nc, ident)
        ones2 = singles.tile([P, ND, P], FP8)
        nc.vector.memset(ones2, ONES_VAL)
        eps_t = singles.tile([P, 1], F32)
        nc.vector.memset(eps_t, EPS_EFF)
        nb2_t = singles.tile([P, 1], F32)
        nc.vector.memset(nb2_t, BIAS_EXP)

        # ---- input DMAs (host pre-packs exact SBUF layouts) ----
        wv_sb = singles.tile([P, ND, H, C], FP8)
        wk_sb = singles.tile([P, ND, H, C], FP8)
        wq_sb = singles.tile([P, ND, H, C], FP8)
        wfc_sb = singles.tile([P, ND, H, C], FP8)
        for w_sb, dram in ((wv_sb, wv8), (wk_sb, wk8), (wq_sb, wq8),
                           (wfc_sb, wfc8)):
            for hh in range(0, H, 4):
                nc.sync.dma_start(out=w_sb[:, :, hh:hh + 4], in_=dram[:, :, hh:hh + 4])
        xt_sb = singles.tile([P, ND, S], FP8)
        for q4 in range(4):
            nc.scalar.dma_start(out=xt_sb[:, :, q4 * 512:(q4 + 1) * 512],
                                in_=xt8[:, :, q4 * 512:(q4 + 1) * 512])
        xqf_sb = singles.tile([P, NR, C], F32)
        nc.scalar.dma_start(out=xqf_sb, in_=xqf[:])
        bqk_sb = singles.tile([P, 2, ND, H], F32)
        nc.scalar.dma_start(out=bqk_sb, in_=bqk[:])
        gb_sb = singles.tile([P, 2, C], F32)
        gb_ap = gb[:]
        gb_bc = bass.AP(tensor=gb_ap.tensor, offset=gb_ap.offset,
                        ap=[[0, P]] + list(gb_ap.ap))
        nc.scalar.dma_start(out=gb_sb, in_=gb_bc)
        gamma_sb = gb_sb[:, 0]
        beta_sb = gb_sb[:, 1]

        # ---- per-head ot output staging (fp8, all heads resident) ----
        ot_all = singles.tile([P, H, NCH, ND, CH], FP8)

        # ---- PE warmup: dummy matmuls while input DMAs land (HAM clock) ----
        warm = prs.tile([P, CH], F32, tag="rs", name="warm")
        for _ in range(48):
            nc.tensor.matmul(warm[:, 0:P], lhsT=ident, rhs=ident,
                             start=True, stop=True)

        # ---- head loop ----
        for h in range(H):
            # V [t, d] projection: 4 t-tiles per psum tile
            v_sb = hpool.tile([P, NT, C], FP8, tag="v")
            for q4 in range(4):
                vps = pst.tile([P, 1024], F32, tag="st", name="vps")
                for i in range(4):
                    t = q4 * 4 + i
                    nc.tensor.matmul(
                        vps[:, i * C:(i + 1) * C],
                        lhsT=xt_sb[:, :, t * P:(t + 1) * P],
                        rhs=wv_sb[:, :, h, :],
                        start=True, stop=True, perf_mode=DR,
                    )
                nc.vector.tensor_copy(out=v_sb[:, q4 * 4:(q4 + 1) * 4, :],
                                      in_=vps)
            # K^T [d, t] projection (+16*bk bias on Pool)
            kt = hpool.tile([P, ND, S], FP8, tag="kt")
            for d2 in range(ND):
                for cc in range(2):
                    kps = pst.tile([P, 1024], F32, tag="st", name="kps")
                    for i in range(2):
                        nc.tensor.matmul(
                            kps[:, i * 512:(i + 1) * 512],
                            lhsT=wk_sb[:, :, h, d2 * P:(d2 + 1) * P],
                            rhs=xt_sb[:, :, (cc * 2 + i) * 512:(cc * 2 + i + 1) * 512],
                            start=True, stop=True, perf_mode=DR,
                        )
                    nc.vector.tensor_scalar_add(
                        out=kt[:, d2, cc * 1024:(cc + 1) * 1024], in0=kps,
                        scalar1=bqk_sb[:, 1, d2, h:h + 1],
                    )
            # Q^T [d, r] projection (+16*bq bias on Pool); query rows are
            # the xt prefix (host rotated the sequence per-core)
            qt = hpool.tile([P, ND, RQ], FP8, tag="qt")
            for d2 in range(ND):
                qps = pst.tile([P, 1024], F32, tag="st", name="qps")
                for i in range(2):
                    nc.tensor.matmul(
                        qps[:, i * 512:(i + 1) * 512],
                        lhsT=wq_sb[:, :, h, d2 * P:(d2 + 1) * P],
                        rhs=xt_sb[:, :, i * 512:(i + 1) * 512],
                        start=True, stop=True, perf_mode=DR,
                    )
                nc.scalar.activation(
                    out=qt[:, d2, :], in_=qps, func=AF.Identity,
                    bias=bqk_sb[:, 0, d2, h:h + 1], scale=1.0,
                )

            # attention, one 512-row chunk at a time
            for ch in range(NCH):
                rsl = slice(ch * CH, (ch + 1) * CH)
                e_sb = epool.tile([P, NT, CH], FP8, tag="e")
                otp = [pot.tile([P, CH], F32, tag="ot", name=f"ot{d2}")
                       for d2 in range(ND)]
                rs = prs.tile([P, CH], F32, tag="rs", name="rs")
                for tp in range(NT // 2):
                    stp = pst.tile([P, 1024], F32, tag="st", name="stp")
                    for i in range(2):
                        t = tp * 2 + i
                        nc.tensor.matmul(
                            stp[:, i * CH:(i + 1) * CH],
                            lhsT=kt[:, :, t * P:(t + 1) * P],
                            rhs=qt[:, :, rsl],
                            start=True, stop=True, perf_mode=DR,
                        )
                    # e = exp(scores/4096 - 2), fp8 out; -2 cancels in softmax
                    nc.scalar.activation(out=e_sb[:, 2 * tp:2 * tp + 2, :],
                                         in_=stp, func=AF.Exp,
                                         scale=float(SCALE_EXP), bias=nb2_t)
                    # rowsum (x0.125) broadcast to 128 partitions
                    nc.tensor.matmul(rs, lhsT=ones2,
                                     rhs=e_sb[:, 2 * tp:2 * tp + 2, :],
                                     start=(tp == 0), stop=(tp == NT // 2 - 1),
                                     perf_mode=DR)
                    for d2 in range(ND):
                        nc.tensor.matmul(
                            otp[d2],
                            lhsT=v_sb[:, 2 * tp:2 * tp + 2, d2 * P:(d2 + 1) * P],
                            rhs=e_sb[:, 2 * tp:2 * tp + 2, :],
                            start=(tp == 0), stop=(tp == NT // 2 - 1),
                            perf_mode=DR,
                        )
                rcp = opool.tile([P, CH], F32, tag="rcp")
                nc.vector.reciprocal_approx_fast(out=rcp, in_=rs)
                for d2 in range(ND):
                    nc.vector.tensor_tensor(
                        out=ot_all[:, h, ch, d2, :], in0=otp[d2], in1=rcp,
                        op=OP.mult)

        # ---- tail: fc over all heads (PSUM-accumulated) + LayerNorm ----
        out_r = out.rearrange("(n p) d -> p n d", p=P)
        for ch in range(NCH):
            for r1 in range(CH // P):
                idx = ch * (CH // P) + r1
                fcp = pst.tile([P, 1024], F32, tag="st", name="fcp")
                for hh in range(H):
                    nc.tensor.matmul(
                        fcp[:, 0:C],
                        lhsT=ot_all[:, hh, ch, :, r1 * P:(r1 + 1) * P],
                        rhs=wfc_sb[:, :, hh, :],
                        start=(hh == 0), stop=(hh == H - 1), perf_mode=DR,
                    )
                u = lnpool.tile([P, C], F32, tag="u", bufs=3)
                # u = fc_psum + 256*(resid + bfc_eff); LN is scale-invariant
                nc.vector.tensor_tensor(out=u, in0=fcp[:, 0:C],
                                        in1=xqf_sb[:, idx], op=OP.add)
                stats = lnpool.tile([P, 6], F32, tag="stats")
                nc.vector.bn_stats(out=stats, in_=u)
                mv = lnpool.tile([P, 2], F32, tag="mv")
                nc.vector.bn_aggr(out=mv, in_=stats)
                sd = lnpool.tile([P, 1], F32, tag="sd")
                nc.scalar.activation(out=sd, in_=mv[:, 1:2], func=AF.Sqrt,
                                     bias=eps_t, scale=1.0)
                rstd = lnpool.tile([P, 1], F32, tag="rstd")
                nc.vector.reciprocal(out=rstd, in_=sd)
                nc.gpsimd.tensor_scalar(out=u, in0=u, scalar1=mv[:, 0:1],
                                        scalar2=rstd, op0=OP.subtract,
                                        op1=OP.mult)
                nc.gpsimd.tensor_tensor(out=u, in0=u, in1=gamma_sb, op=OP.mult)
                nc.gpsimd.tensor_tensor(out=u, in0=u, in1=beta_sb, op=OP.add)
                nc.gpsimd.dma_start(out=out_r[:, idx:idx + 1, :], in_=u)

    nc.finalize()
    return nc


_NC = None


def _get_nc():
    global _NC
    if _NC is None:
        _NC = build_nc()
    return _NC


def make_in_maps(inputs):
    import ml_dtypes
    f8 = ml_dtypes.float8_e4m3
    x = np.asarray(inputs["x"], dtype=np.float32)
    wq = np.asarray(inputs["Wq"], np.float32) * WQK_SCALE
    wk = np.asarray(inputs["Wk"], np.float32) * WQK_SCALE
    wv = np.asarray(inputs["Wv"], np.float32)
    wfc = np.asarray(inputs["Wfc"], np.float32)
    bfc_eff = (np.asarray(inputs["bfc"], np.float32).ravel()
               + np.asarray(inputs["bv"], np.float32).ravel() @ wfc)

    def pack_w(w):  # [H, C, C] -> [P, ND, H, C]
        return np.ascontiguousarray(
            w.reshape(H, ND, P, C).transpose(2, 1, 0, 3).astype(f8))

    shared = {
        "wq8": pack_w(wq),
        "wk8": pack_w(wk),
        "wv8": pack_w(wv),
        "wfc8": pack_w((wfc * WFC_SCALE).reshape(H, C, C)),
        "bqk": np.ascontiguousarray(np.stack([
            (np.asarray(inputs["bq"], np.float32) * WQK_SCALE)
            .reshape(H, ND, P).transpose(2, 1, 0),
            (np.asarray(inputs["bk"], np.float32) * WQK_SCALE)
            .reshape(H, ND, P).transpose(2, 1, 0),
        ], axis=1)),
        "gb": np.ascontiguousarray(np.stack([
            np.asarray(inputs["gamma"], np.float32).ravel(),
            np.asarray(inputs["beta"], np.float32).ravel(),
        ])),
    }
    in_maps = []
    for core in range(8):
        b, r0 = core // 2, (core % 2) * RQ
        m = dict(shared)
        xr = np.roll(x[b], -r0, axis=0)  # query rows first
        m["xt8"] = np.ascontiguousarray(
            xr.T.reshape(ND, P, S).transpose(1, 0, 2).astype(f8))
        m["xqf"] = np.ascontiguousarray(
            ((x[b, r0:r0 + RQ] + bfc_eff[None, :]) * RES_SCALE)
            .reshape(NR, P, C).transpose(1, 0, 2))
        in_maps.append(m)
    return in_maps


def assemble(results):
    out = np.empty((B, S, C), dtype=np.float32)
    for core in range(8):
        b, r0 = core // 2, (core % 2) * RQ
        out[b, r0:r0 + RQ] = results[core]["out"].reshape(RQ, C)
    return out


def kernel(**inputs) -> np.ndarray:
    from concourse.bass_utils import run_bass_kernel_spmd

    nc = _get_nc()
    in_maps = make_in_maps(inputs)
    res = run_bass_kernel_spmd(nc, in_maps, core_ids=list(range(8)))
    return assemble(res.results)


# revision 9
# speedup vs baseline: 1.5311x; 1.0272x over previous
"""Trainium2 Bass kernel for nn_MultiHeadAttention (B=4, S=2048, C=256, H=8).

Sharding: data-parallel over (batch, seq) — 8 cores, core i handles
batch b = i//2 and query rows r0 = (i%2)*1024 .. r0+1024.  Each core
computes K/V projections for its full batch sequence (all 8 heads),
attention + fc for its 1024 query rows, then residual + LayerNorm.
No collectives needed; host concatenates the 8 row-shards.

v2: fp8 (e4m3) DoubleRow matmuls everywhere (2x PE throughput).  The
final output is dominated by the residual (attention contribution has
sigma ~0.036 vs residual sigma 1), so fp8 noise in the attention path
is suppressed ~30x in the output — rel err lands ~1e-3 vs the 2e-2
gate.  Scaling scheme so every fp8 tensor has sigma ~O(1):
  Wq,Wk x16 host-side  -> q,k sigma 16; exp scale 1/4096
  exp bias -2          -> e in [~0, 240] fits e4m3 range (max 240)
  rowsum ones = 0.125  -> ot = 8 * (softmax @ v), sigma ~0.29
  Wfc x32 host-side    -> fc psum = 256 * fc_out
  residual+bias x256 host-side, LN eps x256^2: LN is scale-invariant
Elementwise work is spread across Pool (projection-psum drain + bias),
ACT (exp), and DVE (normalize + LayerNorm) so no single engine
bottlenecks.  fc accumulates over all 8 heads in PSUM at the tail
(per-head ot tiles stay resident in SBUF, 16KB/partition fp8).

Every DMA writes a persistent SBUF buffer or a freshly-allocated pool
tile so each DMA instruction needs at most one semaphore wait.
"""

import sys

for _p in ("/opt/trn_rl_repo",):
    if _p not in sys.path:
        sys.path.insert(0, _p)

from contextlib import ExitStack

import numpy as np

import concourse.bass as bass
from concourse import bacc
import concourse.tile as tile
from concourse import mybir
from concourse.masks import make_identity

P = 128
B, S, C, H = 4, 2048, 256, 8
RQ = 1024            # query rows per core
CH = 512             # query-row chunk (matmul N)
NCH = RQ // CH       # chunks per core = 2
NT = S // P          # t tiles = 16
ND = C // P          # d tiles = 2
NR = RQ // P         # row tiles per core = 8
EPS = 1e-5

WQK_SCALE = 16.0     # host premultiplier on Wq, Wk, bq, bk
WFC_SCALE = 32.0     # host premultiplier on Wfc
RES_SCALE = 256.0    # host premultiplier on residual + bfc_eff
SCALE_EXP = 1.0 / (np.sqrt(C) * WQK_SCALE * WQK_SCALE)  # = 1/4096
BIAS_EXP = -2.0      # keeps exp() below e4m3 max (240); cancels in softmax
ONES_VAL = 0.125     # rowsum lhsT value -> ot = 8 * head_out
EPS_EFF = float(EPS * RES_SCALE * RES_SCALE)

F32 = mybir.dt.float32
BF16 = mybir.dt.bfloat16
FP8 = mybir.dt.float8e4
AF = mybir.ActivationFunctionType
OP = mybir.AluOpType
DR = mybir.MatmulPerfMode.DoubleRow


def build_nc() -> bass.Bass:
    nc = bacc.Bacc(None)

    xt8 = nc.declare_dram_parameter("xt8", [P, ND, S], FP8, isOutput=False)
    xqf = nc.declare_dram_parameter("xqf", [P, NR, C], F32, isOutput=False)
    wq8 = nc.declare_dram_parameter("wq8", [P, ND, H, C], FP8, isOutput=False)
    wk8 = nc.declare_dram_parameter("wk8", [P, ND, H, C], FP8, isOutput=False)
    wv8 = nc.declare_dram_parameter("wv8", [P, ND, H, C], FP8, isOutput=False)
    wfc8 = nc.declare_dram_parameter("wfc8", [P, ND, H, C], FP8, isOutput=False)
    # bqk = host-packed [P, 2, ND, H]: [p, 0] = 16*bq, [p, 1] = 16*bk
    bqk = nc.declare_dram_parameter("bqk", [P, 2, ND, H], F32, isOutput=False)
    gb = nc.declare_dram_parameter("gb", [2, C], F32, isOutput=False)
    out = nc.declare_dram_parameter("out", [RQ, C], F32, isOutput=True)

    with tile.TileContext(nc) as tc, ExitStack() as ctx:
        singles = ctx.enter_context(tc.tile_pool(name="singles", bufs=1))
        hpool = ctx.enter_context(tc.tile_pool(name="hpool", bufs=2))
        epool = ctx.enter_context(tc.tile_pool(name="epool", bufs=2))
        opool = ctx.enter_context(tc.tile_pool(name="opool", bufs=2))
        lnpool = ctx.enter_context(tc.tile_pool(name="lnpool", bufs=4))

        # PSUM: pst 2x[128,1024] = 4 banks, pot 3x[128,512] = 3, prs 1 = 1
        pst = ctx.enter_context(tc.tile_pool(name="pst", bufs=2, space="PSUM"))
        pot = ctx.enter_context(tc.tile_pool(name="pot", bufs=3, space="PSUM"))
        prs = ctx.enter_context(tc.tile_pool(name="prs", bufs=1, space="PSUM"))

        # ---- constants ----
        ident = singles.tile([P, P], BF16)
        make_identity(nc, ident)
        ones2 = singles.tile([P, ND, P], FP8)
        nc.vector.memset(ones2, ONES_VAL)
        eps_t = singles.tile([P, 1], F32)
        nc.vector.memset(eps_t, EPS_EFF)
        nb2_t = singles.tile([P, 1], F32)
        nc.vector.memset(nb2_t, BIAS_EXP)

        # ---- input DMAs (host pre-packs exact SBUF layouts) ----
        wv_sb = singles.tile([P, ND, H, C], FP8)
        wk_sb = singles.tile([P, ND, H, C], FP8)
        wq_sb = singles.tile([P, ND, H, C], FP8)
        wfc_sb = singles.tile([P, ND, H, C], FP8)
        for w_sb, dram in ((wv_sb, wv8), (wk_sb, wk8), (wq_sb, wq8),
                           (wfc_sb, wfc8)):
            for hh in range(0, H, 4):
                nc.sync.dma_start(out=w_sb[:, :, hh:hh + 4], in_=dram[:, :, hh:hh + 4])
        xt_sb = singles.tile([P, ND, S], FP8)
        for q4 in range(4):
            nc.scalar.dma_start(out=xt_sb[:, :, q4 * 512:(q4 + 1) * 512],
                                in_=xt8[:, :, q4 * 512:(q4 + 1) * 512])
        xqf_sb = singles.tile([P, NR, C], F32)
        nc.scalar.dma_start(out=xqf_sb, in_=xqf[:])
        bqk_sb = singles.tile([P, 2, ND, H], F32)
        nc.scalar.dma_start(out=bqk_sb, in_=bqk[:])
        gb_sb = singles.tile([P, 2, C], F32)
        gb_ap = gb[:]
        gb_bc = bass.AP(tensor=gb_ap.tensor, offset=gb_ap.offset,
                        ap=[[0, P]] + list(gb_ap.ap))
        nc.scalar.dma_start(out=gb_sb, in_=gb_bc)
        gamma_sb = gb_sb[:, 0]
        beta_sb = gb_sb[:, 1]

        # ---- per-head ot output staging (fp8, all heads resident) ----
        ot_all = singles.tile([P, H, NCH, ND, CH], FP8)

        # ---- PE warmup: dummy matmuls while input DMAs land (HAM clock) ----
        warm = prs.tile([P, CH], F32, tag="rs", name="warm")
        for _ in range(48):
            nc.tensor.matmul(warm[:, 0:P], lhsT=ident, rhs=ident,
                             start=True, stop=True)

        out_r = out.rearrange("(n p) d -> p n d", p=P)

        # tail for one chunk: fc over all heads (PSUM-accumulated) + LN
        def emit_tail(ch):
            for r1 in range(CH // P):
                idx = ch * (CH // P) + r1
                fcp = pst.tile([P, 1024], F32, tag="st", name="fcp")
                for hh in range(H):
                    nc.tensor.matmul(
                        fcp[:, 0:C],
                        lhsT=ot_all[:, hh, ch, :, r1 * P:(r1 + 1) * P],
                        rhs=wfc_sb[:, :, hh, :],
                        start=(hh == 0), stop=(hh == H - 1), perf_mode=DR,
                    )
                u = lnpool.tile([P, C], F32, tag="u", bufs=3)
                # u = fc_psum + 256*(resid + bfc_eff); LN is scale-invariant
                nc.vector.tensor_tensor(out=u, in0=fcp[:, 0:C],
                                        in1=xqf_sb[:, idx], op=OP.add)
                stats = lnpool.tile([P, 6], F32, tag="stats")
                nc.vector.bn_stats(out=stats, in_=u)
                mv = lnpool.tile([P, 2], F32, tag="mv")
                nc.vector.bn_aggr(out=mv, in_=stats)
                sd = lnpool.tile([P, 1], F32, tag="sd")
                nc.scalar.activation(out=sd, in_=mv[:, 1:2], func=AF.Sqrt,
                                     bias=eps_t, scale=1.0)
                rstd = lnpool.tile([P, 1], F32, tag="rstd")
                nc.vector.reciprocal(out=rstd, in_=sd)
                nc.gpsimd.tensor_scalar(out=u, in0=u, scalar1=mv[:, 0:1],
                                        scalar2=rstd, op0=OP.subtract,
                                        op1=OP.mult)
                nc.gpsimd.tensor_tensor(out=u, in0=u, in1=gamma_sb, op=OP.mult)
                nc.gpsimd.tensor_tensor(out=u, in0=u, in1=beta_sb, op=OP.add)
                nc.gpsimd.dma_start(out=out_r[:, idx:idx + 1, :], in_=u)

        # ---- head loop ----
        for h in range(H):
            # Projections, ordered so PSUM drains (all on DVE) land in the
            # order the attention loop consumes them: Q, K[cols 0:1024],
            # V[t 0:8], K[cols 1024:2048], V[t 8:16].
            qt = hpool.tile([P, ND, RQ], FP8, tag="qt")
            kt = hpool.tile([P, ND, S], FP8, tag="kt")
            v_sb = hpool.tile([P, NT, C], FP8, tag="v")

            def emit_q(d2):
                qps = pst.tile([P, 1024], F32, tag="st", name="qps")
                for i in range(2):
                    nc.tensor.matmul(
                        qps[:, i * 512:(i + 1) * 512],
                        lhsT=wq_sb[:, :, h, d2 * P:(d2 + 1) * P],
                        rhs=xt_sb[:, :, i * 512:(i + 1) * 512],
                        start=True, stop=True, perf_mode=DR,
                    )
                nc.vector.tensor_scalar_add(
                    out=qt[:, d2, :], in0=qps,
                    scalar1=bqk_sb[:, 0, d2, h:h + 1])

            def emit_k(d2, cc):
                kps = pst.tile([P, 1024], F32, tag="st", name="kps")
                for i in range(2):
                    nc.tensor.matmul(
                        kps[:, i * 512:(i + 1) * 512],
                        lhsT=wk_sb[:, :, h, d2 * P:(d2 + 1) * P],
                        rhs=xt_sb[:, :, (cc * 2 + i) * 512:(cc * 2 + i + 1) * 512],
                        start=True, stop=True, perf_mode=DR,
                    )
                nc.vector.tensor_scalar_add(
                    out=kt[:, d2, cc * 1024:(cc + 1) * 1024], in0=kps,
                    scalar1=bqk_sb[:, 1, d2, h:h + 1])

            def emit_v(q4):
                vps = pst.tile([P, 1024], F32, tag="st", name="vps")
                for i in range(4):
                    t = q4 * 4 + i
                    nc.tensor.matmul(
                        vps[:, i * C:(i + 1) * C],
                        lhsT=xt_sb[:, :, t * P:(t + 1) * P],
                        rhs=wv_sb[:, :, h, :],
                        start=True, stop=True, perf_mode=DR,
                    )
                nc.vector.tensor_copy(out=v_sb[:, q4 * 4:(q4 + 1) * 4, :],
                                      in_=vps)

            emit_q(0)
            emit_q(1)
            emit_k(0, 0)
            emit_k(1, 0)
            emit_v(0)
            emit_v(1)
            emit_k(0, 1)
            emit_k(1, 1)
            emit_v(2)
            emit_v(3)

            # attention, one 512-row chunk at a time.  PE stream is software
            # pipelined: scores for pair tp+1 are emitted before the AV/rowsum
            # matmuls of pair tp, so the PE never waits on the ACT exp.
            for ch in range(NCH):
                rsl = slice(ch * CH, (ch + 1) * CH)
                e_sb = epool.tile([P, NT, CH], FP8, tag="e")
                otp = [pot.tile([P, CH], F32, tag="ot", name=f"ot{d2}")
                       for d2 in range(ND)]
                rs = prs.tile([P, CH], F32, tag="rs", name="rs")

                def emit_scores(tp):
                    stp = pst.tile([P, 1024], F32, tag="st", name="stp")
                    for i in range(2):
                        t = tp * 2 + i
                        nc.tensor.matmul(
                            stp[:, i * CH:(i + 1) * CH],
                            lhsT=kt[:, :, t * P:(t + 1) * P],
                            rhs=qt[:, :, rsl],
                            start=True, stop=True, perf_mode=DR,
                        )
                    # e = exp(scores/4096 - 2), fp8; -2 cancels in softmax
                    nc.scalar.activation(out=e_sb[:, 2 * tp:2 * tp + 2, :],
                                         in_=stp, func=AF.Exp,
                                         scale=float(SCALE_EXP), bias=nb2_t)

                def emit_av(tp):
                    nc.tensor.matmul(rs, lhsT=ones2,
                                     rhs=e_sb[:, 2 * tp:2 * tp + 2, :],
                                     start=(tp == 0), stop=(tp == NT // 2 - 1),
                                     perf_mode=DR)
                    for d2 in range(ND):
                        nc.tensor.matmul(
                            otp[d2],
                            lhsT=v_sb[:, 2 * tp:2 * tp + 2, d2 * P:(d2 + 1) * P],
                            rhs=e_sb[:, 2 * tp:2 * tp + 2, :],
                            start=(tp == 0), stop=(tp == NT // 2 - 1),
                            perf_mode=DR,
                        )

                emit_scores(0)
                for tp in range(1, NT // 2):
                    emit_scores(tp)
                    emit_av(tp - 1)
                emit_av(NT // 2 - 1)

                rcp = opool.tile([P, CH], F32, tag="rcp")
                nc.vector.reciprocal_approx_fast(out=rcp, in_=rs)
                for d2 in range(ND):
                    nc.vector.tensor_tensor(
                        out=ot_all[:, h, ch, d2, :], in0=otp[d2], in1=rcp,
                        op=OP.mult)
                # chunk-0 tail overlaps the last head's chunk-1 attention
                if h == H - 1:
                    emit_tail(ch)

    nc.finalize()
    return nc


_NC = None


def _get_nc():
    global _NC
    if _NC is None:
        _NC = build_nc()
    return _NC


def make_in_maps(inputs):
    import ml_dtypes
    f8 = ml_dtypes.float8_e4m3
    x = np.asarray(inputs["x"], dtype=np.float32)
    wq = np.asarray(inputs["Wq"], np.float32) * WQK_SCALE
    wk = np.asarray(inputs["Wk"], np.float32) * WQK_SCALE
    wv = np.asarray(inputs["Wv"], np.float32)
    wfc = np.asarray(inputs["Wfc"], np.float32)
    bfc_eff = (np.asarray(inputs["bfc"], np.float32).ravel()
               + np.asarray(inputs["bv"], np.float32).ravel() @ wfc)

    def pack_w(w):  # [H, C, C] -> [P, ND, H, C]
        return np.ascontiguousarray(
            w.reshape(H, ND, P, C).transpose(2, 1, 0, 3).astype(f8))

    shared = {
        "wq8": pack_w(wq),
        "wk8": pack_w(wk),
        "wv8": pack_w(wv),
        "wfc8": pack_w((wfc * WFC_SCALE).reshape(H, C, C)),
        "bqk": np.ascontiguousarray(np.stack([
            (np.asarray(inputs["bq"], np.float32) * WQK_SCALE)
            .reshape(H, ND, P).transpose(2, 1, 0),
            (np.asarray(inputs["bk"], np.float32) * WQK_SCALE)
            .reshape(H, ND, P).transpose(2, 1, 0),
        ], axis=1)),
        "gb": np.ascontiguousarray(np.stack([
            np.asarray(inputs["gamma"], np.float32).ravel(),
            np.asarray(inputs["beta"], np.float32).ravel(),
        ])),
    }
    in_maps = []
    for core in range(8):
        b, r0 = core // 2, (core % 2) * RQ
        m = dict(shared)
        xr = np.roll(x[b], -r0, axis=0)  # query rows first
        m["xt8"] = np.ascontiguousarray(
            xr.T.reshape(ND, P, S).transpose(1, 0, 2).astype(f8))
        m["xqf"] = np.ascontiguousarray(
            ((x[b, r0:r0 + RQ] + bfc_eff[None, :]) * RES_SCALE)
            .reshape(NR, P, C).transpose(1, 0, 2))
        in_maps.append(m)
    return in_maps


def assemble(results):
    out = np.empty((B, S, C), dtype=np.float32)
    for core in range(8):
        b, r0 = core // 2, (core % 2) * RQ
        out[b, r0:r0 + RQ] = results[core]["out"].reshape(RQ, C)
    return out


def kernel(**inputs) -> np.ndarray:
    from concourse.bass_utils import run_bass_kernel_spmd

    nc = _get_nc()
    in_maps = make_in_maps(inputs)
    res = run_bass_kernel_spmd(nc, in_maps, core_ids=list(range(8)))
    return assemble(res.results)


# revision 12
# speedup vs baseline: 1.7451x; 1.1397x over previous
"""Trainium2 Bass kernel for nn_MultiHeadAttention (B=4, S=2048, C=256, H=8).

Sharding: data-parallel over (batch, seq) — 8 cores, core i handles
batch b = i//2 and query rows r0 = (i%2)*1024 .. r0+1024.  Each core
computes K/V projections for its full batch sequence (all 8 heads),
attention + fc for its 1024 query rows, then residual + LayerNorm.
No collectives needed; host concatenates the 8 row-shards.

v2: fp8 (e4m3) DoubleRow matmuls everywhere (2x PE throughput).  The
final output is dominated by the residual (attention contribution has
sigma ~0.036 vs residual sigma 1), so fp8 noise in the attention path
is suppressed ~30x in the output — rel err lands ~1e-3 vs the 2e-2
gate.  Scaling scheme so every fp8 tensor has sigma ~O(1):
  Wq,Wk x16 host-side  -> q,k sigma 16; exp scale 1/4096
  exp bias -2          -> e in [~0, 240] fits e4m3 range (max 240)
  rowsum ones = 0.125  -> ot = 8 * (softmax @ v), sigma ~0.29
  Wfc x32 host-side    -> fc psum = 256 * fc_out
  residual+bias x256 host-side, LN eps x256^2: LN is scale-invariant
Elementwise work is spread across Pool (projection-psum drain + bias),
ACT (exp), and DVE (normalize + LayerNorm) so no single engine
bottlenecks.  fc accumulates over all 8 heads in PSUM at the tail
(per-head ot tiles stay resident in SBUF, 16KB/partition fp8).

Every DMA writes a persistent SBUF buffer or a freshly-allocated pool
tile so each DMA instruction needs at most one semaphore wait.
"""

import sys

for _p in ("/opt/trn_rl_repo",):
    if _p not in sys.path:
        sys.path.insert(0, _p)

from contextlib import ExitStack

import numpy as np

import concourse.bass as bass
from concourse import bacc
import concourse.tile as tile
from concourse import mybir
from concourse.masks import make_identity

P = 128
B, S, C, H = 4, 2048, 256, 8
RQ = 1024            # query rows per core
CH = 512             # query-row chunk (matmul N)
NCH = RQ // CH       # chunks per core = 2
NT = S // P          # t tiles = 16
ND = C // P          # d tiles = 2
NR = RQ // P         # row tiles per core = 8
EPS = 1e-5

WQK_SCALE = 16.0     # host premultiplier on Wq, Wk, bq, bk
WFC_SCALE = 32.0     # host premultiplier on Wfc
RES_SCALE = 256.0    # host premultiplier on residual + bfc_eff
SCALE_EXP = 1.0 / (np.sqrt(C) * WQK_SCALE * WQK_SCALE)  # = 1/4096
BIAS_EXP = -2.0      # keeps exp() below e4m3 max (240); cancels in softmax
ONES_VAL = 0.125     # rowsum lhsT value -> ot = 8 * head_out
EPS_EFF = float(EPS * RES_SCALE * RES_SCALE)

F32 = mybir.dt.float32
BF16 = mybir.dt.bfloat16
FP8 = mybir.dt.float8e4
AF = mybir.ActivationFunctionType
OP = mybir.AluOpType
DR = mybir.MatmulPerfMode.DoubleRow


def build_nc() -> bass.Bass:
    nc = bacc.Bacc(None)

    xt8 = nc.declare_dram_parameter("xt8", [P, ND, S], FP8, isOutput=False)
    xqf = nc.declare_dram_parameter("xqf", [P, NR, C], F32, isOutput=False)
    wq8 = nc.declare_dram_parameter("wq8", [P, ND, H, C], FP8, isOutput=False)
    wk8 = nc.declare_dram_parameter("wk8", [P, ND, H, C], FP8, isOutput=False)
    wv8 = nc.declare_dram_parameter("wv8", [P, ND, H, C], FP8, isOutput=False)
    wfc8 = nc.declare_dram_parameter("wfc8", [P, ND, H, C], FP8, isOutput=False)
    # bqk = host-packed [P, 2, ND, H]: [p, 0] = 16*bq, [p, 1] = 16*bk
    bqk = nc.declare_dram_parameter("bqk", [P, 2, ND, H], F32, isOutput=False)
    gb = nc.declare_dram_parameter("gb", [2, C], F32, isOutput=False)
    out = nc.declare_dram_parameter("out", [RQ, C], F32, isOutput=True)

    with tile.TileContext(nc) as tc, ExitStack() as ctx:
        singles = ctx.enter_context(tc.tile_pool(name="singles", bufs=1))
        hpool = ctx.enter_context(tc.tile_pool(name="hpool", bufs=2))
        epool = ctx.enter_context(tc.tile_pool(name="epool", bufs=2))
        opool = ctx.enter_context(tc.tile_pool(name="opool", bufs=2))
        lnpool = ctx.enter_context(tc.tile_pool(name="lnpool", bufs=4))

        # PSUM: pst 2x[128,1024] = 4 banks, pot 3x[128,512] = 3, prs 1 = 1
        pst = ctx.enter_context(tc.tile_pool(name="pst", bufs=2, space="PSUM"))
        pot = ctx.enter_context(tc.tile_pool(name="pot", bufs=3, space="PSUM"))
        prs = ctx.enter_context(tc.tile_pool(name="prs", bufs=1, space="PSUM"))

        # ---- constants ----
        ident = singles.tile([P, P], BF16)
        make_identity(nc, ident)
        ones2 = singles.tile([P, ND, P], FP8)
        nc.vector.memset(ones2, ONES_VAL)
        eps_t = singles.tile([P, 1], F32)
        nc.vector.memset(eps_t, EPS_EFF)
        nb2_t = singles.tile([P, 1], F32)
        nc.vector.memset(nb2_t, BIAS_EXP)

        # ---- input DMAs (host pre-packs exact SBUF layouts) ----
        wv_sb = singles.tile([P, ND, H, C], FP8)
        wk_sb = singles.tile([P, ND, H, C], FP8)
        wq_sb = singles.tile([P, ND, H, C], FP8)
        wfc_sb = singles.tile([P, ND, H, C], FP8)
        # head 0+1 weight slices land first so compute starts ~1.5us in
        for hh in ((0, 2), (2, 8)):
            for w_sb, dram in ((wq_sb, wq8), (wk_sb, wk8), (wv_sb, wv8)):
                nc.sync.dma_start(out=w_sb[:, :, hh[0]:hh[1]],
                                  in_=dram[:, :, hh[0]:hh[1]])
        nc.sync.dma_start(out=wfc_sb, in_=wfc8[:])
        xt_sb = singles.tile([P, ND, S], FP8)
        for q4 in range(4):
            nc.scalar.dma_start(out=xt_sb[:, :, q4 * 512:(q4 + 1) * 512],
                                in_=xt8[:, :, q4 * 512:(q4 + 1) * 512])
        bqk_sb = singles.tile([P, 2, ND, H], F32)
        nc.scalar.dma_start(out=bqk_sb, in_=bqk[:])
        xqf_sb = singles.tile([P, NR, C], F32)
        nc.scalar.dma_start(out=xqf_sb, in_=xqf[:])
        gb_sb = singles.tile([P, 2, C], F32)
        gb_ap = gb[:]
        gb_bc = bass.AP(tensor=gb_ap.tensor, offset=gb_ap.offset,
                        ap=[[0, P]] + list(gb_ap.ap))
        nc.scalar.dma_start(out=gb_sb, in_=gb_bc)
        gamma_sb = gb_sb[:, 0]
        beta_sb = gb_sb[:, 1]

        # ---- per-head ot output staging (fp8, all heads resident) ----
        ot_all = singles.tile([P, H, NCH, ND, CH], FP8)

        # ---- PE warmup: dummy matmuls while input DMAs land (HAM clock) ----
        warm = prs.tile([P, CH], F32, tag="rs", name="warm")
        for _ in range(48):
            nc.tensor.matmul(warm[:, 0:P], lhsT=ident, rhs=ident,
                             start=True, stop=True)

        out_r = out.rearrange("(n p) d -> p n d", p=P)

        # tail for one chunk: fc over all heads (PSUM-accumulated) + LN
        def emit_tail(ch):
            for r1 in range(CH // P):
                idx = ch * (CH // P) + r1
                fcp = pst.tile([P, 1024], F32, tag="st", name="fcp")
                for hh in range(H):
                    nc.tensor.matmul(
                        fcp[:, 0:C],
                        lhsT=ot_all[:, hh, ch, :, r1 * P:(r1 + 1) * P],
                        rhs=wfc_sb[:, :, hh, :],
                        start=(hh == 0), stop=(hh == H - 1), perf_mode=DR,
                    )
                u = lnpool.tile([P, C], F32, tag="u", bufs=3)
                # u = fc_psum + 256*(resid + bfc_eff); LN is scale-invariant
                nc.vector.tensor_tensor(out=u, in0=fcp[:, 0:C],
                                        in1=xqf_sb[:, idx], op=OP.add)
                stats = lnpool.tile([P, 6], F32, tag="stats")
                nc.vector.bn_stats(out=stats, in_=u)
                mv = lnpool.tile([P, 2], F32, tag="mv")
                nc.vector.bn_aggr(out=mv, in_=stats)
                sd = lnpool.tile([P, 1], F32, tag="sd")
                nc.scalar.activation(out=sd, in_=mv[:, 1:2], func=AF.Sqrt,
                                     bias=eps_t, scale=1.0)
                rstd = lnpool.tile([P, 1], F32, tag="rstd")
                nc.vector.reciprocal(out=rstd, in_=sd)
                nc.vector.tensor_scalar(out=u, in0=u, scalar1=mv[:, 0:1],
                                        scalar2=rstd, op0=OP.subtract,
                                        op1=OP.mult)
                nc.gpsimd.tensor_tensor(out=u, in0=u, in1=gamma_sb, op=OP.mult)
                nc.gpsimd.tensor_tensor(out=u, in0=u, in1=beta_sb, op=OP.add)
                nc.gpsimd.dma_start(out=out_r[:, idx:idx + 1, :], in_=u)

        # ---- head loop ----
        for h in range(H):
            # Projections, ordered so PSUM drains (all on DVE) land in the
            # order the attention loop consumes them: Q, K[cols 0:1024],
            # V[t 0:8], K[cols 1024:2048], V[t 8:16].
            qt = hpool.tile([P, ND, RQ], FP8, tag="qt")
            kt = hpool.tile([P, ND, S], FP8, tag="kt")
            v_sb = hpool.tile([P, NT, C], FP8, tag="v")

            def emit_q(d2):
                qps = pst.tile([P, 1024], F32, tag="st", name="qps")
                for i in range(2):
                    nc.tensor.matmul(
                        qps[:, i * 512:(i + 1) * 512],
                        lhsT=wq_sb[:, :, h, d2 * P:(d2 + 1) * P],
                        rhs=xt_sb[:, :, i * 512:(i + 1) * 512],
                        start=True, stop=True, perf_mode=DR,
                    )
                nc.scalar.activation(
                    out=qt[:, d2, :], in_=qps, func=AF.Identity,
                    bias=bqk_sb[:, 0, d2, h:h + 1], scale=1.0)

            def emit_k(d2, cc):
                kps = pst.tile([P, 1024], F32, tag="st", name="kps")
                for i in range(2):
                    nc.tensor.matmul(
                        kps[:, i * 512:(i + 1) * 512],
                        lhsT=wk_sb[:, :, h, d2 * P:(d2 + 1) * P],
                        rhs=xt_sb[:, :, (cc * 2 + i) * 512:(cc * 2 + i + 1) * 512],
                        start=True, stop=True, perf_mode=DR,
                    )
                nc.vector.tensor_scalar_add(
                    out=kt[:, d2, cc * 1024:(cc + 1) * 1024], in0=kps,
                    scalar1=bqk_sb[:, 1, d2, h:h + 1])

            def emit_v(q4):
                vps = pst.tile([P, 1024], F32, tag="st", name="vps")
                for i in range(4):
                    t = q4 * 4 + i
                    nc.tensor.matmul(
                        vps[:, i * C:(i + 1) * C],
                        lhsT=xt_sb[:, :, t * P:(t + 1) * P],
                        rhs=wv_sb[:, :, h, :],
                        start=True, stop=True, perf_mode=DR,
                    )
                nc.vector.tensor_copy(out=v_sb[:, q4 * 4:(q4 + 1) * 4, :],
                                      in_=vps)

            emit_q(0)
            emit_q(1)
            emit_k(0, 0)
            emit_k(1, 0)
            emit_v(0)
            emit_v(1)
            emit_k(0, 1)
            emit_k(1, 1)
            emit_v(2)
            emit_v(3)

            # attention, one 512-row chunk at a time.  PE stream is software
            # pipelined: scores for pair tp+1 are emitted before the AV/rowsum
            # matmuls of pair tp, so the PE never waits on the ACT exp.
            for ch in range(NCH):
                rsl = slice(ch * CH, (ch + 1) * CH)
                e_sb = epool.tile([P, NT, CH], FP8, tag="e")
                otp = [pot.tile([P, CH], F32, tag="ot", name=f"ot{d2}")
                       for d2 in range(ND)]
                rs = prs.tile([P, CH], F32, tag="rs", name="rs")

                def emit_scores(tp):
                    stp = pst.tile([P, 1024], F32, tag="st", name="stp")
                    for i in range(2):
                        t = tp * 2 + i
                        nc.tensor.matmul(
                            stp[:, i * CH:(i + 1) * CH],
                            lhsT=kt[:, :, t * P:(t + 1) * P],
                            rhs=qt[:, :, rsl],
                            start=True, stop=True, perf_mode=DR,
                        )
                    # e = exp(scores/4096 - 2), fp8; -2 cancels in softmax
                    nc.scalar.activation(out=e_sb[:, 2 * tp:2 * tp + 2, :],
                                         in_=stp, func=AF.Exp,
                                         scale=float(SCALE_EXP), bias=nb2_t)

                def emit_av(tp):
                    nc.tensor.matmul(rs, lhsT=ones2,
                                     rhs=e_sb[:, 2 * tp:2 * tp + 2, :],
                                     start=(tp == 0), stop=(tp == NT // 2 - 1),
                                     perf_mode=DR)
                    for d2 in range(ND):
                        nc.tensor.matmul(
                            otp[d2],
                            lhsT=v_sb[:, 2 * tp:2 * tp + 2, d2 * P:(d2 + 1) * P],
                            rhs=e_sb[:, 2 * tp:2 * tp + 2, :],
                            start=(tp == 0), stop=(tp == NT // 2 - 1),
                            perf_mode=DR,
                        )

                emit_scores(0)
                for tp in range(1, NT // 2):
                    emit_scores(tp)
                    emit_av(tp - 1)
                emit_av(NT // 2 - 1)

                rcp = opool.tile([P, CH], F32, tag="rcp")
                nc.vector.reciprocal_approx_fast(out=rcp, in_=rs)
                for d2 in range(ND):
                    nc.vector.tensor_tensor(
                        out=ot_all[:, h, ch, d2, :], in0=otp[d2], in1=rcp,
                        op=OP.mult)
                # chunk-0 tail overlaps the last head's chunk-1 attention
                if h == H - 1:
                    emit_tail(ch)

    nc.finalize()
    return nc


_NC = None


def _get_nc():
    global _NC
    if _NC is None:
        _NC = build_nc()
    return _NC


def make_in_maps(inputs):
    import ml_dtypes
    f8 = ml_dtypes.float8_e4m3
    x = np.asarray(inputs["x"], dtype=np.float32)
    wq = np.asarray(inputs["Wq"], np.float32) * WQK_SCALE
    wk = np.asarray(inputs["Wk"], np.float32) * WQK_SCALE
    wv = np.asarray(inputs["Wv"], np.float32)
    wfc = np.asarray(inputs["Wfc"], np.float32)
    bfc_eff = (np.asarray(inputs["bfc"], np.float32).ravel()
               + np.asarray(inputs["bv"], np.float32).ravel() @ wfc)

    def pack_w(w):  # [H, C, C] -> [P, ND, H, C]
        return np.ascontiguousarray(
            w.reshape(H, ND, P, C).transpose(2, 1, 0, 3).astype(f8))

    shared = {
        "wq8": pack_w(wq),
        "wk8": pack_w(wk),
        "wv8": pack_w(wv),
        "wfc8": pack_w((wfc * WFC_SCALE).reshape(H, C, C)),
        "bqk": np.ascontiguousarray(np.stack([
            (np.asarray(inputs["bq"], np.float32) * WQK_SCALE)
            .reshape(H, ND, P).transpose(2, 1, 0),
            (np.asarray(inputs["bk"], np.float32) * WQK_SCALE)
            .reshape(H, ND, P).transpose(2, 1, 0),
        ], axis=1)),
        "gb": np.ascontiguousarray(np.stack([
            np.asarray(inputs["gamma"], np.float32).ravel(),
            np.asarray(inputs["beta"], np.float32).ravel(),
        ])),
    }
    in_maps = []
    for core in range(8):
        b, r0 = core // 2, (core % 2) * RQ
        m = dict(shared)
        xr = np.roll(x[b], -r0, axis=0)  # query rows first
        m["xt8"] = np.ascontiguousarray(
            xr.T.reshape(ND, P, S).transpose(1, 0, 2).astype(f8))
        m["xqf"] = np.ascontiguousarray(
            ((x[b, r0:r0 + RQ] + bfc_eff[None, :]) * RES_SCALE)
            .reshape(NR, P, C).transpose(1, 0, 2))
        in_maps.append(m)
    return in_maps


def assemble(results):
    out = np.empty((B, S, C), dtype=np.float32)
    for core in range(8):
        b, r0 = core // 2, (core % 2) * RQ
        out[b, r0:r0 + RQ] = results[core]["out"].reshape(RQ, C)
    return out


def kernel(**inputs) -> np.ndarray:
    from concourse.bass_utils import run_bass_kernel_spmd

    nc = _get_nc()
    in_maps = make_in_maps(inputs)
    res = run_bass_kernel_spmd(nc, in_maps, core_ids=list(range(8)))
    return assemble(res.results)
